# revision 7
# baseline (speedup 1.0000x reference)
"""BiAttentionLayer Trainium2 kernel (Bass/Tile), data-parallel over batch N.

Full inputs:  H [64,1024,200], U [64,64,200], c_mask [64,1024],
              q_mask [64,64], w [600], b []
Full output:  G [64,1024,800] = concat([H, U_, H*U_, H*H_], -1)

Sharding: batch rows 8 per core across 8 NeuronCores; masks/w/b replicated.

Math notes (exactly matches the reference up to fp rounding):
  S = (H@w_h)[:,:,None] + (U@w_u)[:,None,:] + (H*w_hu)@U^T + b
  masked_softmax(v,m) simplifies to exp(v*m)*m / sum_j(exp(v*m)*m) because
  the inner softmax normalizer cancels on renormalization (eps 1e-13 is
  negligible at fp32).  We compute e = exp((S*qm) + NEG*(1-qm)) with
  NEG=-100 via  e = exp((Sq_partial + (S1+b+100))*qm - 100)  so one ACT op
  yields both the masked numerator (masked lanes ~3.7e-44 ~ 0) and, via
  accum_out, the denominator.  The Q2C path needs exp(max_j S_rep)*cm,
  which equals max_j(e)*cm, so no second exp/softmax is required; the
  (T-long) softmax normalizer is folded into the PE-accumulated weighted
  sum via an extra ones column.
"""

import os
import sys

for _p in ("/opt/trn_rl_repo", "/root/.axon_site/_ro/trn_rl_repo"):
    if os.path.isdir(_p) and _p not in sys.path:
        sys.path.insert(0, _p)

import numpy as np

import concourse.bass as bass
import concourse.tile as tile
from concourse import mybir
from concourse.masks import make_identity

N_CORES = 8
N_FULL = 64
B = N_FULL // N_CORES          # batch rows per core
T = 1024
J = 64
D2 = 200
DG = 4 * D2                    # 800
NCHUNK = T // 128              # 8
K1, K2 = 128, D2 - 128         # contraction split 128 + 72
NEG_SOFT = 100.0               # exp(x - 100): masked lanes underflow to ~0

FP = mybir.dt.float32
MULT = mybir.AluOpType.mult
ADD = mybir.AluOpType.add


def _split_overwide_waits(nc, max_waits=1):
    """This walrus build only encodes one semaphore wait per instruction;
    hoist extra waits onto no-ops just before the offending instruction."""
    for bb in nc.m.functions[0].blocks:
        i = 0
        while i < len(bb.instructions):
            ins = bb.instructions[i]
            si = getattr(ins, "sync_info", None)
            if si is not None and si.on_wait is not None and len(si.on_wait) > max_waits:
                waits = list(si.on_wait)
                si.on_wait = waits[-max_waits:]
                rest = waits[:-max_waits]
                k = 0
                while rest:
                    chunk, rest = rest[:max_waits], rest[max_waits:]
                    nop = mybir.InstNoOp(
                        name=f"{ins.name}-wsplit{k}",
                        engine=ins.engine,
                        bass_nofuse=True,
                        sync_info=mybir.SyncInfo(on_wait=chunk, on_update=[]),
                    )
                    bb.instructions.insert(i, nop)
                    i += 1
                    k += 1
            i += 1


def build_program(split_waits=True):
    nc = bass.Bass()

    H_d = nc.dram_tensor("H", [B, T, D2], FP, kind="ExternalInput")
    U_d = nc.dram_tensor("U", [B, J, D2], FP, kind="ExternalInput")
    cm_d = nc.dram_tensor("c_mask", [B, T], FP, kind="ExternalInput")
    qm_d = nc.dram_tensor("q_mask", [B, J], FP, kind="ExternalInput")
    w_d = nc.dram_tensor("w", [3 * D2], FP, kind="ExternalInput")
    b_d = nc.dram_tensor("b", [1, 1], FP, kind="ExternalInput")
    G_d = nc.dram_tensor("G", [B, T, DG], FP, kind="ExternalOutput")

    with tile.TileContext(nc) as tc:
        with (
            tc.tile_pool(name="const", bufs=1) as constp,
            tc.tile_pool(name="row", bufs=2) as rowp,
            tc.tile_pool(name="chunk", bufs=3) as chp,
            tc.tile_pool(name="gbuf", bufs=2 * NCHUNK) as gp,
            tc.tile_pool(name="ps_tr", bufs=2, space="PSUM") as ps_trp,
            tc.tile_pool(name="ps_s", bufs=2, space="PSUM") as ps_sp,
            tc.tile_pool(name="ps_u", bufs=2, space="PSUM") as ps_up,
            tc.tile_pool(name="ps_hbar", bufs=1, space="PSUM") as ps_hbarp,
            tc.tile_pool(name="ps_hb", bufs=1, space="PSUM") as ps_hbp,
        ):
            # ---- constants ----
            ident = constp.tile([128, 128], FP)
            make_identity(nc, ident)
            ones_row = constp.tile([1, 128], FP)
            nc.vector.memset(ones_row, 1.0)
            ones_col = constp.tile([128, 1], FP)
            nc.vector.memset(ones_col, 1.0)
            negc = constp.tile([128, 1], FP)
            nc.vector.memset(negc, -NEG_SOFT)
            b_sb = constp.tile([1, 1], FP)
            nc.gpsimd.dma_start(out=b_sb, in_=b_d[:, :])
            # w chunks as [K,1] columns on partitions
            wh1 = constp.tile([K1, 1], FP)
            wh2 = constp.tile([K2, 1], FP)
            wu1 = constp.tile([K1, 1], FP)
            wu2 = constp.tile([K2, 1], FP)
            whu1 = constp.tile([K1, 1], FP)
            whu2 = constp.tile([K2, 1], FP)
            for sb, lo in ((wh1, 0), (wh2, K1), (wu1, D2), (wu2, D2 + K1),
                           (whu1, 2 * D2), (whu2, 2 * D2 + K1)):
                n = sb.shape[0]
                nc.gpsimd.dma_start(out=sb, in_=w_d[lo:lo + n].unsqueeze(1))

            for r in range(B):
                # ---------------- row setup ----------------
                U_sb = rowp.tile([J, D2], FP)
                nc.sync.dma_start(out=U_sb, in_=U_d[r])
                qm_b = rowp.tile([128, J], FP)
                nc.gpsimd.dma_start(out=qm_b, in_=qm_d[r].partition_broadcast(128))
                cm_t = rowp.tile([128, NCHUNK], FP)
                nc.gpsimd.dma_start(
                    out=cm_t, in_=cm_d[r].rearrange("(c p) -> p c", p=128)
                )

                # U^T via PE transpose (two D2 chunks), S2 = U@w_u, S2q row
                tru = ps_trp.tile([128, 384], FP, tag="tr")
                nc.tensor.transpose(tru[0:K1, 0:J], U_sb[:, 0:K1], ident[0:J, 0:J])
                nc.tensor.transpose(
                    tru[0:K2, J:2 * J], U_sb[:, K1:D2], ident[0:J, 0:J]
                )
                ut1 = rowp.tile([K1, J], FP)
                ut2 = rowp.tile([K2, J], FP)
                nc.scalar.copy(out=ut1, in_=tru[0:K1, 0:J])
                nc.scalar.copy(out=ut2, in_=tru[0:K2, J:2 * J])

                # rhs for the S matmul: cols 0:64 = U^T * w_hu * qm, col 64 = w_h
                uwq1 = rowp.tile([K1, J + 1], FP)
                uwq2 = rowp.tile([K2, J + 1], FP)
                nc.vector.scalar_tensor_tensor(
                    out=uwq1[:, 0:J], in0=ut1, scalar=whu1[:, 0:1],
                    in1=qm_b[0:K1, :], op0=MULT, op1=MULT,
                )
                nc.vector.scalar_tensor_tensor(
                    out=uwq2[:, 0:J], in0=ut2, scalar=whu2[:, 0:1],
                    in1=qm_b[0:K2, :], op0=MULT, op1=MULT,
                )
                nc.vector.tensor_copy(out=uwq1[:, J:J + 1], in_=wh1)
                nc.vector.tensor_copy(out=uwq2[:, J:J + 1], in_=wh2)

                # S2 = U @ w_u  -> [J,1] -> transpose -> s2q row [1, 65]
                nc.tensor.matmul(tru[0:J, 128:129], ut1, wu1, start=True, stop=False)
                nc.tensor.matmul(tru[0:J, 128:129], ut2, wu2, start=False, stop=True)
                s2col = rowp.tile([J, 1], FP)
                nc.vector.tensor_copy(out=s2col, in_=tru[0:J, 128:129])
                nc.tensor.transpose(tru[0:1, 136:200], s2col, ident[0:J, 0:J])
                s2q = rowp.tile([1, J + 1], FP)
                nc.vector.tensor_tensor(
                    out=s2q[:, 0:J], in0=tru[0:1, 136:200], in1=qm_b[0:1, :], op=MULT
                )
                nc.vector.tensor_copy(out=s2q[:, J:J + 1], in_=b_sb)

                denoms = rowp.tile([128, NCHUNK], FP)
                rt = rowp.tile([128, NCHUNK], FP)
                hbar = ps_hbarp.tile([1, D2 + 1], FP)
                g_tiles = []

                # ---------------- pass 1 over T-chunks ----------------
                for c in range(NCHUNK):
                    t0 = c * 128
                    g_c = gp.tile([128, DG], FP, tag="g")
                    g_tiles.append(g_c)
                    nc.sync.dma_start(out=g_c[:, 0:D2], in_=H_d[r, t0:t0 + 128, :])
                    # transient ones column for the fused Hbar numerator+denom
                    # matmul; overwritten by the U_ copy below
                    nc.vector.memset(g_c[:, D2:D2 + 1], 1.0)

                    # H^T chunks (PE transpose + copy out of PSUM)
                    trc = ps_trp.tile([128, 384], FP, tag="tr")
                    nc.tensor.transpose(trc[:, 0:128], g_c[:, 0:K1], ident)
                    nc.tensor.transpose(trc[0:K2, 128:256], g_c[:, K1:D2], ident)
                    ht1 = chp.tile([K1, 128], FP)
                    ht2 = chp.tile([K2, 128], FP)
                    nc.scalar.copy(out=ht1, in_=trc[:, 0:128])
                    nc.scalar.copy(out=ht2, in_=trc[0:K2, 128:256])

                    # S matmul: cols 0:64 = (S3+S2)*qm, col 64 = S1 + b
                    ps_s = ps_sp.tile([128, J + 1], FP)
                    nc.tensor.matmul(ps_s, ht1, uwq1, start=True, stop=False)
                    nc.tensor.matmul(ps_s, ht2, uwq2, start=False, stop=False)
                    nc.tensor.matmul(ps_s, ones_row, s2q, start=False, stop=True)

                    # s1b = S1 + b + 100 ;  vmq = (Sq + s1b)*qm ;  e = exp(vmq-100)
                    s1b = chp.tile([128, 1], FP)
                    nc.vector.tensor_scalar_add(
                        out=s1b, in0=ps_s[:, J:J + 1], scalar1=NEG_SOFT
                    )
                    vmq = chp.tile([128, J], FP)
                    nc.vector.scalar_tensor_tensor(
                        out=vmq, in0=ps_s[:, 0:J], scalar=s1b[:, 0:1],
                        in1=qm_b, op0=ADD, op1=MULT,
                    )
                    e_c = chp.tile([128, J], FP)
                    nc.scalar.activation(
                        out=e_c, in_=vmq, func=mybir.ActivationFunctionType.Exp,
                        bias=negc[:, 0:1], scale=1.0,
                        accum_out=denoms[:, c:c + 1],
                    )

                    # Q2C numerator: rt = max_j(e) * cm
                    maxe = chp.tile([128, 1], FP)
                    nc.vector.reduce_max(maxe, e_c, axis=mybir.AxisListType.X)
                    nc.vector.tensor_tensor(
                        out=rt[:, c:c + 1], in0=maxe, in1=cm_t[:, c:c + 1], op=MULT
                    )

                    # S_t = e * (1/denom)
                    rden = chp.tile([128, 1], FP)
                    nc.vector.reciprocal(out=rden, in_=denoms[:, c:c + 1])
                    st_c = chp.tile([128, J], FP)
                    nc.vector.tensor_scalar_mul(out=st_c, in0=e_c, scalar1=rden[:, 0:1])

                    # U_ = S_t @ U : transpose S_t then matmul with U rows
                    nc.tensor.transpose(trc[0:J, 256:384], st_c, ident)
                    stT = chp.tile([J, 128], FP)
                    nc.vector.tensor_copy(out=stT, in_=trc[0:J, 256:384])
                    ps_u = ps_up.tile([128, D2], FP)
                    nc.tensor.matmul(ps_u, stT, U_sb, start=True, stop=True)

                    # Hbar += rt_c @ [H | 1]  (single accumulation chain per bank)
                    nc.tensor.matmul(
                        hbar, rt[:, c:c + 1], g_c[:, 0:D2 + 1],
                        start=(c == 0), stop=(c == NCHUNK - 1),
                    )

                    # G columns: U_ copy then H*U_
                    nc.scalar.copy(out=g_c[:, D2:2 * D2], in_=ps_u)
                    nc.vector.tensor_tensor(
                        out=g_c[:, 2 * D2:3 * D2], in0=g_c[:, 0:D2],
                        in1=g_c[:, D2:2 * D2], op=MULT,
                    )

                # ---------------- row finalize: H_ and pass 2 ----------------
                rs = rowp.tile([1, 1], FP)
                nc.vector.tensor_scalar_add(
                    out=rs, in0=hbar[:, D2:D2 + 1], scalar1=1e-13
                )
                nc.vector.reciprocal(out=rs, in_=rs)
                hbar_sb = rowp.tile([1, D2], FP)
                nc.vector.tensor_scalar_mul(
                    out=hbar_sb, in0=hbar[:, 0:D2], scalar1=rs[:, 0:1]
                )
                ps_hb = ps_hbp.tile([128, D2], FP)
                nc.tensor.matmul(ps_hb, ones_row, hbar_sb, start=True, stop=True)

                for c in range(NCHUNK):
                    g_c = g_tiles[c]
                    nc.vector.tensor_tensor(
                        out=g_c[:, 3 * D2:4 * D2], in0=g_c[:, 0:D2],
                        in1=ps_hb, op=MULT,
                    )
                    t0 = c * 128
                    nc.sync.dma_start(out=G_d[r, t0:t0 + 128, :], in_=g_c)

    if split_waits:
        _split_overwide_waits(nc)
    return nc


_NC_CACHE = None


def _get_nc():
    global _NC_CACHE
    if _NC_CACHE is None:
        _NC_CACHE = build_program()
    return _NC_CACHE


def run_sharded(inputs, trace=False):
    from concourse.bass_utils import run_bass_kernel_spmd

    H = np.ascontiguousarray(np.asarray(inputs["H"], dtype=np.float32))
    U = np.ascontiguousarray(np.asarray(inputs["U"], dtype=np.float32))
    cm = np.ascontiguousarray(np.asarray(inputs["c_mask"], dtype=np.float32))
    qm = np.ascontiguousarray(np.asarray(inputs["q_mask"], dtype=np.float32))
    w = np.ascontiguousarray(np.asarray(inputs["w"], dtype=np.float32))
    b = np.asarray(inputs["b"], dtype=np.float32).reshape(1, 1)

    nc = _get_nc()
    in_maps = []
    for c in range(N_CORES):
        s = slice(c * B, (c + 1) * B)
        in_maps.append(
            {"H": H[s], "U": U[s], "c_mask": cm[s], "q_mask": qm[s], "w": w, "b": b}
        )
    res = run_bass_kernel_spmd(
        nc, in_maps, core_ids=list(range(N_CORES)), trace=trace
    )
    G = np.concatenate([res.results[c]["G"] for c in range(N_CORES)], axis=0)
    return G, res


def kernel(H, U, c_mask, q_mask, w, b):
    G, _ = run_sharded(
        {"H": H, "U": U, "c_mask": c_mask, "q_mask": q_mask, "w": w, "b": b}
    )
    return G


# revision 17
# speedup vs baseline: 1.5247x; 1.5247x over previous
"""BiAttentionLayer Trainium2 kernel (Bass/Tile), data-parallel over batch N.

Full inputs:  H [64,1024,200], U [64,64,200], c_mask [64,1024],
              q_mask [64,64], w [600], b []
Full output:  G [64,1024,800] = concat([H, U_, H*U_, H*H_], -1)

Sharding: batch rows 8 per core across 8 NeuronCores; masks/w/b replicated.

Math (matches the reference to fp rounding):
  S = (H@w_h)[:,:,None] + (U@w_u)[:,None,:] + (H*w_hu)@U^T + b
  masked_softmax(v,m) == exp(v*m)*m / sum_j(exp(v*m)*m)   (normalizer of the
  inner softmax cancels on renormalization; the 1e-13 eps is negligible).
  With NEG=100:  e = exp((Sq_cols + (S1+b+NEG))*qm - NEG)  gives the masked
  numerator in one ACT op (masked lanes underflow to ~0), so
    denom_t = sum_j e,   S_t = e/denom,   exp(S_max)*cm = max_j(e)*cm.
  U_ = (e @ U) * (1/denom)  — normalization folded into the PSUM->SBUF copy.
  H_ = (rt @ [H|1]) with rt = max_j(e)*cm, normalized by the ones column.

Schedule: chunk pipeline head(c)=DMA+transpose+S-matmul (PE-dense),
soft(c)=masked softmax (ACT/DVE), tail(c)=e-transpose+U_ matmul; emitted
as head(c+2) / soft(c+1) / tail(c) so ~3 chunks are in flight; the H_
reduction runs as one back-to-back PE burst at row end.
"""

import os
import sys

for _p in ("/opt/trn_rl_repo", "/root/.axon_site/_ro/trn_rl_repo"):
    if os.path.isdir(_p) and _p not in sys.path:
        sys.path.insert(0, _p)

import numpy as np

import concourse.bass as bass
import concourse.tile as tile
from concourse import mybir
from concourse.masks import make_identity

N_CORES = 8
N_FULL = 64
B = N_FULL // N_CORES          # batch rows per core
T = 1024
J = 64
D2 = 200
DG = 4 * D2                    # 800
NCHUNK = T // 128              # 8
K1, K2 = 128, D2 - 128         # contraction split 128 + 72
NEG_SOFT = 100.0               # exp(x - 100): masked lanes underflow to ~0

FP = mybir.dt.float32
MULT = mybir.AluOpType.mult
ADD = mybir.AluOpType.add
AXX = mybir.AxisListType.X
EXP = mybir.ActivationFunctionType.Exp
COPYF = mybir.ActivationFunctionType.Copy


def _split_overwide_waits(nc, max_waits=1):
    """This walrus build only encodes one semaphore wait per instruction;
    hoist extra waits onto no-ops just before the offending instruction."""
    for bb in nc.m.functions[0].blocks:
        i = 0
        while i < len(bb.instructions):
            ins = bb.instructions[i]
            si = getattr(ins, "sync_info", None)
            if si is not None and si.on_wait is not None and len(si.on_wait) > max_waits:
                waits = list(si.on_wait)
                si.on_wait = waits[-max_waits:]
                rest = waits[:-max_waits]
                k = 0
                while rest:
                    chunk, rest = rest[:max_waits], rest[max_waits:]
                    nop = mybir.InstNoOp(
                        name=f"{ins.name}-wsplit{k}",
                        engine=ins.engine,
                        bass_nofuse=True,
                        sync_info=mybir.SyncInfo(on_wait=chunk, on_update=[]),
                    )
                    bb.instructions.insert(i, nop)
                    i += 1
                    k += 1
            i += 1


def build_program(split_waits=True):
    nc = bass.Bass()

    H_d = nc.dram_tensor("H", [B, T, D2], FP, kind="ExternalInput")
    U_d = nc.dram_tensor("U", [B, J, D2], FP, kind="ExternalInput")
    cm_d = nc.dram_tensor("c_mask", [B, T], FP, kind="ExternalInput")
    qm_d = nc.dram_tensor("q_mask", [B, J], FP, kind="ExternalInput")
    w_d = nc.dram_tensor("w", [3 * D2], FP, kind="ExternalInput")
    b_d = nc.dram_tensor("b", [1, 1], FP, kind="ExternalInput")
    G_d = nc.dram_tensor("G", [B, T, DG], FP, kind="ExternalOutput")

    with tile.TileContext(nc) as tc:
        with (
            tc.tile_pool(name="const", bufs=1) as constp,
            tc.tile_pool(name="row", bufs=2) as rowp,
            tc.tile_pool(name="chunk", bufs=4) as chp,
            tc.tile_pool(name="gbuf", bufs=2 * NCHUNK) as gp,
            tc.tile_pool(name="ps_tr", bufs=2, space="PSUM") as ps_trp,
            tc.tile_pool(name="ps_s", bufs=3, space="PSUM") as ps_sp,
            tc.tile_pool(name="ps_u", bufs=3, space="PSUM") as ps_up,
        ):
            # ---- constants ----
            ident = constp.tile([128, 128], FP)
            make_identity(nc, ident)
            ones_row = constp.tile([1, 128], FP)
            nc.vector.memset(ones_row, 1.0)
            negc = constp.tile([128, 1], FP)
            nc.vector.memset(negc, -NEG_SOFT)
            b_sb = constp.tile([1, 1], FP)
            nc.gpsimd.dma_start(out=b_sb, in_=b_d[:, :])
            b100 = constp.tile([1, 1], FP)
            nc.vector.tensor_scalar_add(out=b100, in0=b_sb, scalar1=NEG_SOFT)
            wh1 = constp.tile([K1, 1], FP)
            wh2 = constp.tile([K2, 1], FP)
            wu1 = constp.tile([K1, 1], FP)
            wu2 = constp.tile([K2, 1], FP)
            whu1 = constp.tile([K1, 1], FP)
            whu2 = constp.tile([K2, 1], FP)
            for sb, lo in ((wh1, 0), (wh2, K1), (wu1, D2), (wu2, D2 + K1),
                           (whu1, 2 * D2), (whu2, 2 * D2 + K1)):
                n = sb.shape[0]
                nc.gpsimd.dma_start(out=sb, in_=w_d[lo:lo + n].unsqueeze(1))

            for r in range(B):
                # ---------------- row setup ----------------
                U_sb = rowp.tile([J, D2], FP)
                nc.sync.dma_start(out=U_sb, in_=U_d[r])
                qm_b = rowp.tile([128, J], FP)
                nc.gpsimd.dma_start(out=qm_b, in_=qm_d[r].partition_broadcast(128))
                cm_t = rowp.tile([128, NCHUNK], FP)
                nc.gpsimd.dma_start(
                    out=cm_t, in_=cm_d[r].rearrange("(c p) -> p c", p=128)
                )

                # U^T via PE transpose (two D2 chunks); S2 = U@w_u
                tru = ps_trp.tile([128, 384], FP, tag="tr")
                nc.tensor.transpose(tru[0:K1, 0:J], U_sb[:, 0:K1], ident[0:J, 0:J])
                nc.tensor.transpose(
                    tru[0:K2, J:2 * J], U_sb[:, K1:D2], ident[0:J, 0:J]
                )
                ut1 = rowp.tile([K1, J], FP)
                ut2 = rowp.tile([K2, J], FP)
                nc.scalar.copy(out=ut1, in_=tru[0:K1, 0:J])
                nc.scalar.copy(out=ut2, in_=tru[0:K2, J:2 * J])

                # S-matmul rhs: uwq1 [128,65] cols j = U^T*w_hu*qm, col 64 = w_h
                # uwq2 [73,65]: rows 0:72 ditto, row 72 = [S2*qm | b+100],
                # matched by an lhsT ones row produced by transposing the
                # memset ones column g_c[:, 200].
                uwq1 = rowp.tile([K1, J + 1], FP)
                uwq2 = rowp.tile([K2 + 1, J + 1], FP)
                nc.vector.scalar_tensor_tensor(
                    out=uwq1[:, 0:J], in0=ut1, scalar=whu1[:, 0:1],
                    in1=qm_b[0:K1, :], op0=MULT, op1=MULT,
                )
                nc.vector.scalar_tensor_tensor(
                    out=uwq2[0:K2, 0:J], in0=ut2, scalar=whu2[:, 0:1],
                    in1=qm_b[0:K2, :], op0=MULT, op1=MULT,
                )
                nc.vector.tensor_copy(out=uwq1[:, J:J + 1], in_=wh1)
                nc.vector.tensor_copy(out=uwq2[0:K2, J:J + 1], in_=wh2)

                nc.tensor.matmul(tru[0:J, 128:129], ut1, wu1, start=True, stop=False)
                nc.tensor.matmul(tru[0:J, 128:129], ut2, wu2, start=False, stop=True)
                s2col = rowp.tile([J, 1], FP)
                nc.vector.tensor_copy(out=s2col, in_=tru[0:J, 128:129])
                nc.tensor.transpose(tru[0:1, 136:200], s2col, ident[0:J, 0:J])
                s2q = rowp.tile([1, J + 1], FP)
                nc.vector.tensor_tensor(
                    out=s2q[:, 0:J], in0=tru[0:1, 136:200],
                    in1=qm_b[0:1, :], op=MULT,
                )
                nc.vector.tensor_copy(out=s2q[:, J:J + 1], in_=b100)
                nc.sync.dma_start(out=uwq2[K2:K2 + 1, :], in_=s2q)

                denoms = rowp.tile([128, NCHUNK], FP)
                maxes = rowp.tile([128, NCHUNK], FP)
                rt = rowp.tile([128, NCHUNK], FP)
                g_tiles = [None] * NCHUNK
                ps_s_t = [None] * NCHUNK
                e_pairs = [None] * (NCHUNK // 2)
                rden_pairs = [None] * (NCHUNK // 2)
                ps_b_t = [None] * NCHUNK

                # ---------------- pipelined chunk stages ----------------
                def head(c):
                    t0 = c * 128
                    g_c = gp.tile([128, DG], FP, tag="g")
                    g_tiles[c] = g_c
                    nc.sync.dma_start(
                        out=g_c[:, 0:D2], in_=H_d[r, t0:t0 + 128, :]
                    )
                    # transient ones column: transposed into the lhsT ones row
                    # for the S2 rank-1 term; overwritten later by U_
                    nc.vector.memset(g_c[:, D2:D2 + 1], 1.0)
                    trc = ps_trp.tile([128, 256], FP, tag="tr")
                    nc.tensor.transpose(trc[:, 0:128], g_c[:, 0:K1], ident)
                    nc.tensor.transpose(
                        trc[0:K2 + 1, 128:256], g_c[:, K1:D2 + 1], ident
                    )
                    ht = chp.tile([128, 256], FP, tag="ht")
                    nc.scalar.copy(out=ht[:, 0:128], in_=trc[:, 0:128])
                    nc.scalar.copy(
                        out=ht[0:K2 + 1, 128:256], in_=trc[0:K2 + 1, 128:256]
                    )
                    ps_s = ps_sp.tile([128, J + 1], FP, tag="s")
                    ps_s_t[c] = ps_s
                    nc.tensor.matmul(
                        ps_s, ht[:, 0:128], uwq1, start=True, stop=False
                    )
                    nc.tensor.matmul(
                        ps_s, ht[0:K2 + 1, 128:256], uwq2, start=False, stop=True
                    )

                def soft(c):
                    ps_s = ps_s_t[c]
                    vmq = chp.tile([128, J], FP, tag="vmq")
                    nc.vector.scalar_tensor_tensor(
                        out=vmq, in0=ps_s[:, 0:J], scalar=ps_s[:, J:J + 1],
                        in1=qm_b, op0=ADD, op1=MULT,
                    )
                    if c % 2 == 0:
                        ep = chp.tile([128, 2 * J], FP, tag="e")
                        e_pairs[c // 2] = ep
                    e_pair = e_pairs[c // 2]
                    half = (c % 2) * J
                    nc.scalar.activation(
                        out=e_pair[:, half:half + J], in_=vmq, func=EXP,
                        bias=negc[:, 0:1], scale=1.0,
                    )
                    if c % 2 == 1:
                        ep3 = e_pair.rearrange("p (k j) -> p k j", j=J)
                        nc.vector.reduce_sum(
                            denoms[:, c - 1:c + 1], ep3, axis=AXX
                        )
                        nc.vector.reduce_max(
                            maxes[:, c - 1:c + 1], ep3, axis=AXX
                        )
                        rp = chp.tile([128, 2], FP, tag="rden")
                        rden_pairs[c // 2] = rp
                        nc.vector.reciprocal(
                            out=rp, in_=denoms[:, c - 1:c + 1]
                        )

                def tail(c):
                    e_pair = e_pairs[c // 2]
                    half = (c % 2) * J
                    ps_a = ps_up.tile([128, D2], FP, tag="u")
                    nc.tensor.transpose(
                        ps_a[0:J, 0:128], e_pair[:, half:half + J], ident
                    )
                    eT = chp.tile([J, 128], FP, tag="eT")
                    nc.vector.tensor_copy(out=eT, in_=ps_a[0:J, 0:128])
                    ps_b = ps_up.tile([128, D2], FP, tag="u")
                    ps_b_t[c] = ps_b
                    nc.tensor.matmul(ps_b, eT, U_sb, start=True, stop=True)
                    # U_ = (e@U) * 1/denom, fused into the PSUM->SBUF copy
                    rp = rden_pairs[c // 2]
                    nc.scalar.activation(
                        out=g_c_of[c][:, D2:2 * D2], in_=ps_b, func=COPYF,
                        scale=rp[:, c % 2:c % 2 + 1],
                    )
                    nc.vector.tensor_tensor(
                        out=g_c_of[c][:, 2 * D2:3 * D2], in0=g_c_of[c][:, 0:D2],
                        in1=g_c_of[c][:, D2:2 * D2], op=MULT,
                    )

                g_c_of = g_tiles  # alias used inside tail()

                head(0)
                head(1)
                soft(0)
                for c in range(NCHUNK):
                    if c + 2 < NCHUNK:
                        head(c + 2)
                    if c + 1 < NCHUNK:
                        soft(c + 1)
                    tail(c)

                # ---------------- row finalize ----------------
                nc.vector.tensor_tensor(out=rt, in0=maxes, in1=cm_t, op=MULT)
                hbar = ps_up.tile([1, D2], FP, tag="u")
                for c in range(NCHUNK):
                    nc.tensor.matmul(
                        hbar, rt[:, c:c + 1], g_tiles[c][:, 0:D2],
                        start=(c == 0), stop=(c == NCHUNK - 1),
                    )
                # rsum = sum(rt) via per-partition reduce + transpose + reduce
                rtp = rowp.tile([128, 1], FP)
                nc.vector.reduce_sum(rtp, rt, axis=AXX)
                trr = ps_trp.tile([1, 128], FP, tag="tr")
                nc.tensor.transpose(trr, rtp, ident)
                rtr = rowp.tile([1, 128], FP)
                nc.vector.tensor_copy(out=rtr, in_=trr)
                rs = rowp.tile([1, 1], FP)
                nc.vector.reduce_sum(rs, rtr, axis=AXX)
                nc.vector.tensor_scalar_add(out=rs, in0=rs, scalar1=1e-13)
                nc.vector.reciprocal(out=rs, in_=rs)
                hbar_sb = rowp.tile([1, D2], FP)
                nc.vector.tensor_scalar_mul(
                    out=hbar_sb, in0=hbar[:, 0:D2], scalar1=rs[:, 0:1]
                )
                ps_hb = ps_up.tile([128, D2], FP, tag="u")
                nc.tensor.matmul(ps_hb, ones_row, hbar_sb, start=True, stop=True)
                hb_sb = rowp.tile([128, D2], FP)
                nc.vector.tensor_copy(out=hb_sb, in_=ps_hb)

                for c in range(NCHUNK):
                    g_c = g_tiles[c]
                    nc.gpsimd.tensor_mul(
                        g_c[:, 3 * D2:4 * D2], g_c[:, 0:D2], hb_sb
                    )
                    t0 = c * 128
                    nc.sync.dma_start(out=G_d[r, t0:t0 + 128, :], in_=g_c)

    if split_waits:
        _split_overwide_waits(nc)
    return nc


_NC_CACHE = None


def _get_nc():
    global _NC_CACHE
    if _NC_CACHE is None:
        _NC_CACHE = build_program()
    return _NC_CACHE


def run_sharded(inputs, trace=False):
    from concourse.bass_utils import run_bass_kernel_spmd

    H = np.ascontiguousarray(np.asarray(inputs["H"], dtype=np.float32))
    U = np.ascontiguousarray(np.asarray(inputs["U"], dtype=np.float32))
    cm = np.ascontiguousarray(np.asarray(inputs["c_mask"], dtype=np.float32))
    qm = np.ascontiguousarray(np.asarray(inputs["q_mask"], dtype=np.float32))
    w = np.ascontiguousarray(np.asarray(inputs["w"], dtype=np.float32))
    b = np.asarray(inputs["b"], dtype=np.float32).reshape(1, 1)

    nc = _get_nc()
    in_maps = []
    for c in range(N_CORES):
        s = slice(c * B, (c + 1) * B)
        in_maps.append(
            {"H": H[s], "U": U[s], "c_mask": cm[s], "q_mask": qm[s], "w": w, "b": b}
        )
    res = run_bass_kernel_spmd(
        nc, in_maps, core_ids=list(range(N_CORES)), trace=trace
    )
    G = np.concatenate([res.results[c]["G"] for c in range(N_CORES)], axis=0)
    return G, res


def kernel(H, U, c_mask, q_mask, w, b):
    G, _ = run_sharded(
        {"H": H, "U": U, "c_mask": c_mask, "q_mask": q_mask, "w": w, "b": b}
    )
    return G


# revision 19
# speedup vs baseline: 1.5641x; 1.0258x over previous
"""BiAttentionLayer Trainium2 kernel (Bass/Tile), data-parallel over batch N.

Full inputs:  H [64,1024,200], U [64,64,200], c_mask [64,1024],
              q_mask [64,64], w [600], b []
Full output:  G [64,1024,800] = concat([H, U_, H*U_, H*H_], -1)

Sharding: batch rows 8 per core across 8 NeuronCores; masks/w/b replicated.

Math (matches the reference to fp rounding):
  S = (H@w_h)[:,:,None] + (U@w_u)[:,None,:] + (H*w_hu)@U^T + b
  masked_softmax(v,m) == exp(v*m)*m / sum_j(exp(v*m)*m)   (normalizer of the
  inner softmax cancels on renormalization; the 1e-13 eps is negligible).
  With NEG=100:  e = exp((Sq_cols + (S1+b+NEG))*qm - NEG)  gives the masked
  numerator in one ACT op (masked lanes underflow to ~0), so
    denom_t = sum_j e,   S_t = e/denom,   exp(S_max)*cm = max_j(e)*cm.
  U_ = (e @ U) * (1/denom)  — normalization folded into the PSUM->SBUF copy.
  H_ = (rt @ [H|1]) with rt = max_j(e)*cm, normalized by the ones column.

Schedule: chunk pipeline head(c)=DMA+transpose+S-matmul (PE-dense),
soft(c)=masked softmax (ACT/DVE), tail(c)=e-transpose+U_ matmul; emitted
as head(c+2) / soft(c+1) / tail(c) so ~3 chunks are in flight; the H_
reduction runs as one back-to-back PE burst at row end.
"""

import os
import sys

for _p in ("/opt/trn_rl_repo", "/root/.axon_site/_ro/trn_rl_repo"):
    if os.path.isdir(_p) and _p not in sys.path:
        sys.path.insert(0, _p)

import numpy as np

import concourse.bass as bass
import concourse.tile as tile
from concourse import mybir
from concourse.masks import make_identity

N_CORES = 8
N_FULL = 64
B = N_FULL // N_CORES          # batch rows per core
T = 1024
J = 64
D2 = 200
DG = 4 * D2                    # 800
NCHUNK = T // 128              # 8
K1, K2 = 128, D2 - 128         # contraction split 128 + 72
NEG_SOFT = 100.0               # exp(x - 100): masked lanes underflow to ~0

FP = mybir.dt.float32
MULT = mybir.AluOpType.mult
ADD = mybir.AluOpType.add
AXX = mybir.AxisListType.X
EXP = mybir.ActivationFunctionType.Exp
COPYF = mybir.ActivationFunctionType.Copy


def _split_overwide_waits(nc, max_waits=1):
    """This walrus build only encodes one semaphore wait per instruction;
    hoist extra waits onto no-ops just before the offending instruction."""
    for bb in nc.m.functions[0].blocks:
        i = 0
        while i < len(bb.instructions):
            ins = bb.instructions[i]
            si = getattr(ins, "sync_info", None)
            if si is not None and si.on_wait is not None and len(si.on_wait) > max_waits:
                waits = list(si.on_wait)
                si.on_wait = waits[-max_waits:]
                rest = waits[:-max_waits]
                k = 0
                while rest:
                    chunk, rest = rest[:max_waits], rest[max_waits:]
                    nop = mybir.InstNoOp(
                        name=f"{ins.name}-wsplit{k}",
                        engine=ins.engine,
                        bass_nofuse=True,
                        sync_info=mybir.SyncInfo(on_wait=chunk, on_update=[]),
                    )
                    bb.instructions.insert(i, nop)
                    i += 1
                    k += 1
            i += 1


def build_program(split_waits=True):
    nc = bass.Bass()

    H_d = nc.dram_tensor("H", [B, T, D2], FP, kind="ExternalInput")
    U_d = nc.dram_tensor("U", [B, J, D2], FP, kind="ExternalInput")
    cm_d = nc.dram_tensor("c_mask", [B, T], FP, kind="ExternalInput")
    qm_d = nc.dram_tensor("q_mask", [B, J], FP, kind="ExternalInput")
    w_d = nc.dram_tensor("w", [3 * D2], FP, kind="ExternalInput")
    b_d = nc.dram_tensor("b", [1, 1], FP, kind="ExternalInput")
    G_d = nc.dram_tensor("G", [B, T, DG], FP, kind="ExternalOutput")

    with tile.TileContext(nc) as tc:
        with (
            tc.tile_pool(name="const", bufs=1) as constp,
            tc.tile_pool(name="row", bufs=2) as rowp,
            tc.tile_pool(name="chunk", bufs=6) as chp,
            tc.tile_pool(name="gbuf", bufs=2 * NCHUNK) as gp,
            tc.tile_pool(name="ps_tr", bufs=2, space="PSUM") as ps_trp,
            tc.tile_pool(name="ps_s", bufs=3, space="PSUM") as ps_sp,
            tc.tile_pool(name="ps_u", bufs=3, space="PSUM") as ps_up,
        ):
            # ---- constants ----
            ident = constp.tile([128, 128], FP)
            make_identity(nc, ident)
            ones_row = constp.tile([1, 128], FP)
            nc.vector.memset(ones_row, 1.0)
            negc = constp.tile([128, 1], FP)
            nc.vector.memset(negc, -NEG_SOFT)
            b_sb = constp.tile([1, 1], FP)
            nc.gpsimd.dma_start(out=b_sb, in_=b_d[:, :])
            b100 = constp.tile([1, 1], FP)
            nc.vector.tensor_scalar_add(out=b100, in0=b_sb, scalar1=NEG_SOFT)
            wh1 = constp.tile([K1, 1], FP)
            wh2 = constp.tile([K2, 1], FP)
            wu1 = constp.tile([K1, 1], FP)
            wu2 = constp.tile([K2, 1], FP)
            whu1 = constp.tile([K1, 1], FP)
            whu2 = constp.tile([K2, 1], FP)
            for sb, lo in ((wh1, 0), (wh2, K1), (wu1, D2), (wu2, D2 + K1),
                           (whu1, 2 * D2), (whu2, 2 * D2 + K1)):
                n = sb.shape[0]
                nc.gpsimd.dma_start(out=sb, in_=w_d[lo:lo + n].unsqueeze(1))

            for r in range(B):
                # ---------------- row setup ----------------
                U_sb = rowp.tile([J, D2], FP)
                nc.sync.dma_start(out=U_sb, in_=U_d[r])
                qm_b = rowp.tile([128, J], FP)
                nc.gpsimd.dma_start(out=qm_b, in_=qm_d[r].partition_broadcast(128))
                cm_t = rowp.tile([128, NCHUNK], FP)
                nc.gpsimd.dma_start(
                    out=cm_t, in_=cm_d[r].rearrange("(c p) -> p c", p=128)
                )

                # U^T via PE transpose (two D2 chunks); S2 = U@w_u
                tru = ps_trp.tile([128, 384], FP, tag="tr")
                nc.tensor.transpose(tru[0:K1, 0:J], U_sb[:, 0:K1], ident[0:J, 0:J])
                nc.tensor.transpose(
                    tru[0:K2, J:2 * J], U_sb[:, K1:D2], ident[0:J, 0:J]
                )
                ut1 = rowp.tile([K1, J], FP)
                ut2 = rowp.tile([K2, J], FP)
                nc.scalar.copy(out=ut1, in_=tru[0:K1, 0:J])
                nc.scalar.copy(out=ut2, in_=tru[0:K2, J:2 * J])

                # S-matmul rhs: uwq1 [128,65] cols j = U^T*w_hu*qm, col 64 = w_h
                # uwq2 [73,65]: rows 0:72 ditto, row 72 = [S2*qm | b+100],
                # matched by an lhsT ones row produced by transposing the
                # memset ones column g_c[:, 200].
                uwq1 = rowp.tile([K1, J + 1], FP)
                uwq2 = rowp.tile([K2 + 1, J + 1], FP)
                nc.vector.scalar_tensor_tensor(
                    out=uwq1[:, 0:J], in0=ut1, scalar=whu1[:, 0:1],
                    in1=qm_b[0:K1, :], op0=MULT, op1=MULT,
                )
                nc.vector.scalar_tensor_tensor(
                    out=uwq2[0:K2, 0:J], in0=ut2, scalar=whu2[:, 0:1],
                    in1=qm_b[0:K2, :], op0=MULT, op1=MULT,
                )
                nc.vector.tensor_copy(out=uwq1[:, J:J + 1], in_=wh1)
                nc.vector.tensor_copy(out=uwq2[0:K2, J:J + 1], in_=wh2)

                nc.tensor.matmul(tru[0:J, 128:129], ut1, wu1, start=True, stop=False)
                nc.tensor.matmul(tru[0:J, 128:129], ut2, wu2, start=False, stop=True)
                s2col = rowp.tile([J, 1], FP)
                nc.vector.tensor_copy(out=s2col, in_=tru[0:J, 128:129])
                nc.tensor.transpose(tru[0:1, 136:200], s2col, ident[0:J, 0:J])
                s2q = rowp.tile([1, J + 1], FP)
                nc.vector.tensor_tensor(
                    out=s2q[:, 0:J], in0=tru[0:1, 136:200],
                    in1=qm_b[0:1, :], op=MULT,
                )
                nc.vector.tensor_copy(out=s2q[:, J:J + 1], in_=b100)
                nc.sync.dma_start(out=uwq2[K2:K2 + 1, :], in_=s2q)

                denoms = rowp.tile([128, NCHUNK], FP)
                maxes = rowp.tile([128, NCHUNK], FP)
                rt = rowp.tile([128, NCHUNK], FP)
                g_tiles = [None] * NCHUNK
                ps_s_t = [None] * NCHUNK
                e_pairs = [None] * (NCHUNK // 2)
                rden_pairs = [None] * (NCHUNK // 2)
                ps_b_t = [None] * NCHUNK

                # ---------------- pipelined chunk stages ----------------
                def head(c):
                    t0 = c * 128
                    g_c = gp.tile([128, DG], FP, tag="g")
                    g_tiles[c] = g_c
                    nc.sync.dma_start(
                        out=g_c[:, 0:D2], in_=H_d[r, t0:t0 + 128, :]
                    )
                    # transient ones column: transposed into the lhsT ones row
                    # for the S2 rank-1 term; overwritten later by U_
                    nc.vector.memset(g_c[:, D2:D2 + 1], 1.0)
                    trc = ps_trp.tile([128, 256], FP, tag="tr")
                    nc.tensor.transpose(trc[:, 0:128], g_c[:, 0:K1], ident)
                    nc.tensor.transpose(
                        trc[0:K2 + 1, 128:256], g_c[:, K1:D2 + 1], ident
                    )
                    ht = chp.tile([128, 256], FP, tag="ht")
                    nc.scalar.copy(out=ht[:, 0:128], in_=trc[:, 0:128])
                    nc.scalar.copy(
                        out=ht[0:K2 + 1, 128:256], in_=trc[0:K2 + 1, 128:256]
                    )
                    ps_s = ps_sp.tile([128, J + 1], FP, tag="s")
                    ps_s_t[c] = ps_s
                    nc.tensor.matmul(
                        ps_s, ht[:, 0:128], uwq1, start=True, stop=False
                    )
                    nc.tensor.matmul(
                        ps_s, ht[0:K2 + 1, 128:256], uwq2, start=False, stop=True
                    )

                def soft(c):
                    ps_s = ps_s_t[c]
                    vmq = chp.tile([128, J], FP, tag="vmq")
                    nc.vector.scalar_tensor_tensor(
                        out=vmq, in0=ps_s[:, 0:J], scalar=ps_s[:, J:J + 1],
                        in1=qm_b, op0=ADD, op1=MULT,
                    )
                    if c % 2 == 0:
                        ep = chp.tile([128, 2 * J], FP, tag="e")
                        e_pairs[c // 2] = ep
                    e_pair = e_pairs[c // 2]
                    half = (c % 2) * J
                    nc.scalar.activation(
                        out=e_pair[:, half:half + J], in_=vmq, func=EXP,
                        bias=negc[:, 0:1], scale=1.0,
                    )
                    if c % 2 == 1:
                        ep3 = e_pair.rearrange("p (k j) -> p k j", j=J)
                        nc.vector.reduce_sum(
                            denoms[:, c - 1:c + 1], ep3, axis=AXX
                        )
                        nc.vector.reduce_max(
                            maxes[:, c - 1:c + 1], ep3, axis=AXX
                        )
                        rp = chp.tile([128, 2], FP, tag="rden")
                        rden_pairs[c // 2] = rp
                        nc.vector.reciprocal(
                            out=rp, in_=denoms[:, c - 1:c + 1]
                        )

                def tail(c):
                    e_pair = e_pairs[c // 2]
                    half = (c % 2) * J
                    ps_a = ps_up.tile([128, D2], FP, tag="u")
                    nc.tensor.transpose(
                        ps_a[0:J, 0:128], e_pair[:, half:half + J], ident
                    )
                    eT = chp.tile([J, 128], FP, tag="eT")
                    nc.vector.tensor_copy(out=eT, in_=ps_a[0:J, 0:128])
                    ps_b = ps_up.tile([128, D2], FP, tag="u")
                    ps_b_t[c] = ps_b
                    nc.tensor.matmul(ps_b, eT, U_sb, start=True, stop=True)
                    # U_ = (e@U) * 1/denom, fused into the PSUM->SBUF copy
                    rp = rden_pairs[c // 2]
                    nc.scalar.activation(
                        out=g_c_of[c][:, D2:2 * D2], in_=ps_b, func=COPYF,
                        scale=rp[:, c % 2:c % 2 + 1],
                    )
                    nc.vector.tensor_tensor(
                        out=g_c_of[c][:, 2 * D2:3 * D2], in0=g_c_of[c][:, 0:D2],
                        in1=g_c_of[c][:, D2:2 * D2], op=MULT,
                    )

                g_c_of = g_tiles  # alias used inside tail()

                head(0)
                head(1)
                head(2)
                soft(0)
                for c in range(NCHUNK):
                    if c + 1 < NCHUNK:
                        soft(c + 1)
                    tail(c)
                    if c + 3 < NCHUNK:
                        head(c + 3)

                # ---------------- row finalize ----------------
                nc.vector.tensor_tensor(out=rt, in0=maxes, in1=cm_t, op=MULT)
                hbar = ps_up.tile([1, D2], FP, tag="u")
                for c in range(NCHUNK):
                    nc.tensor.matmul(
                        hbar, rt[:, c:c + 1], g_tiles[c][:, 0:D2],
                        start=(c == 0), stop=(c == NCHUNK - 1),
                    )
                # rsum = sum(rt) via per-partition reduce + transpose + reduce
                rtp = rowp.tile([128, 1], FP)
                nc.vector.reduce_sum(rtp, rt, axis=AXX)
                trr = ps_trp.tile([1, 128], FP, tag="tr")
                nc.tensor.transpose(trr, rtp, ident)
                rtr = rowp.tile([1, 128], FP)
                nc.vector.tensor_copy(out=rtr, in_=trr)
                rs = rowp.tile([1, 1], FP)
                nc.vector.reduce_sum(rs, rtr, axis=AXX)
                nc.vector.tensor_scalar_add(out=rs, in0=rs, scalar1=1e-13)
                nc.vector.reciprocal(out=rs, in_=rs)
                hbar_sb = rowp.tile([1, D2], FP)
                nc.vector.tensor_scalar_mul(
                    out=hbar_sb, in0=hbar[:, 0:D2], scalar1=rs[:, 0:1]
                )
                ps_hb = ps_up.tile([128, D2], FP, tag="u")
                nc.tensor.matmul(ps_hb, ones_row, hbar_sb, start=True, stop=True)
                hb_sb = rowp.tile([128, D2], FP)
                nc.vector.tensor_copy(out=hb_sb, in_=ps_hb)

                for c in range(NCHUNK):
                    g_c = g_tiles[c]
                    nc.gpsimd.tensor_mul(
                        g_c[:, 3 * D2:4 * D2], g_c[:, 0:D2], hb_sb
                    )
                    t0 = c * 128
                    nc.sync.dma_start(out=G_d[r, t0:t0 + 128, :], in_=g_c)

    if split_waits:
        _split_overwide_waits(nc)
    return nc


_NC_CACHE = None


def _get_nc():
    global _NC_CACHE
    if _NC_CACHE is None:
        _NC_CACHE = build_program()
    return _NC_CACHE


def run_sharded(inputs, trace=False):
    from concourse.bass_utils import run_bass_kernel_spmd

    H = np.ascontiguousarray(np.asarray(inputs["H"], dtype=np.float32))
    U = np.ascontiguousarray(np.asarray(inputs["U"], dtype=np.float32))
    cm = np.ascontiguousarray(np.asarray(inputs["c_mask"], dtype=np.float32))
    qm = np.ascontiguousarray(np.asarray(inputs["q_mask"], dtype=np.float32))
    w = np.ascontiguousarray(np.asarray(inputs["w"], dtype=np.float32))
    b = np.asarray(inputs["b"], dtype=np.float32).reshape(1, 1)

    nc = _get_nc()
    in_maps = []
    for c in range(N_CORES):
        s = slice(c * B, (c + 1) * B)
        in_maps.append(
            {"H": H[s], "U": U[s], "c_mask": cm[s], "q_mask": qm[s], "w": w, "b": b}
        )
    res = run_bass_kernel_spmd(
        nc, in_maps, core_ids=list(range(N_CORES)), trace=trace
    )
    G = np.concatenate([res.results[c]["G"] for c in range(N_CORES)], axis=0)
    return G, res


def kernel(H, U, c_mask, q_mask, w, b):
    G, _ = run_sharded(
        {"H": H, "U": U, "c_mask": c_mask, "q_mask": q_mask, "w": w, "b": b}
    )
    return G
